# revision 3
# baseline (speedup 1.0000x reference)
"""Multi-head causal attention (B=2, S=2048, E=1024, H=16, D=64) on 8 TRN2
NeuronCores.

Sharding (data + tensor parallel, Megatron-style):
  core c -> batch b = c // 4, head group g = c % 4 (4 heads, e' = 256 cols).
  Wq/Wk/Wv column-sharded ([256, 1024] slices), Wo row-sharded
  ([1024, 256] slice); each core produces a partial output [2048, 1024]
  (f16) which the host sums per batch group (the Megatron all-reduce) and
  adds bo.

Per-core device kernel (matmul operands fp16, accumulate fp32 in PSUM),
restructured for ACT(exp)-stream continuity:
  - inputs DMA'd in 512-column blocks so K-projection starts ~8us in and
    the first exp fires ~20us in (xq blocks descending: Q tile tt=3 first).
  - attention in S^T orientation per (q-tile 512, head-pair chunk c):
    batches of 8 k-tiles: logits pairs (2 heads row-tiled on the PE at
    partitions 0-63/64-127), exp on ACT (1/8 scale folded), triangular
    trimming: diagonal tiles only compute columns >= dd and apply a single
    [128,2,128] lower-tri mask; attnV accumulates V'^T @ P^T into PSUM
    [65, 512] (row 64 = softmax denominator via V' ones column), lagging
    the exp stream by 2 tiles.
  - V-projection tiles and the previous q-tile's O-projection are woven
    into the attention stream at batch boundaries so the PE's spare
    capacity is used while ACT streams exp.
  - normalize: DVE reciprocal on the PSUM denominator row, GpSimd
    partition-broadcast, DVE multiply into valsT (no transpose DMAs).
"""
import sys
import os

sys.path.insert(0, "/opt/trn_rl_repo")

import numpy as np
from contextlib import ExitStack

import concourse.bass as bass  # noqa: E402
import concourse.mybir as mybir  # noqa: E402
import concourse.tile as tile  # noqa: E402
from concourse import bacc, bass_utils  # noqa: E402

bass_utils.upload_artifacts = lambda d: f"local:{d}"

B, S, E, H, D = 2, 2048, 1024, 16, 64
NCORES = 8
EL = 256  # e' columns per core (4 heads)
F32 = mybir.dt.float32
F16 = mybir.dt.float16
AF = mybir.ActivationFunctionType
NP16 = np.float16

_CACHE = {}


def _build():
    nc = bacc.Bacc("TRN2", target_bir_lowering=False, debug=False)

    xq_d = nc.dram_tensor("xqT", [E, S], F16, kind="ExternalInput")
    xk_d = nc.dram_tensor("xkT", [E, S], F16, kind="ExternalInput")
    xv_d = nc.dram_tensor("xvT", [E, S], F16, kind="ExternalInput")
    wq_d = nc.dram_tensor("wqT", [E, EL], F16, kind="ExternalInput")
    wk_d = nc.dram_tensor("wkT", [E, EL], F16, kind="ExternalInput")
    wv_d = nc.dram_tensor("wvT", [E, EL], F16, kind="ExternalInput")
    wo_d = nc.dram_tensor("woT", [EL, E], F16, kind="ExternalInput")
    bq_d = nc.dram_tensor("bq", [EL], F32, kind="ExternalInput")
    bk_d = nc.dram_tensor("bk", [EL], F32, kind="ExternalInput")
    bv_d = nc.dram_tensor("bv", [EL], F32, kind="ExternalInput")
    vones_d = nc.dram_tensor("vones", [128, 16, 4, 1], F16, kind="ExternalInput")
    mask_d = nc.dram_tensor("masks", [128, 2, 128], F16, kind="ExternalInput")
    out_d = nc.dram_tensor("out", [S, E], F16, kind="ExternalOutput")

    with tile.TileContext(nc) as tc, ExitStack() as ctx:
        cpool = ctx.enter_context(tc.tile_pool(name="const", bufs=1))
        psp = ctx.enter_context(tc.tile_pool(name="psp", bufs=2, space="PSUM"))
        expp = ctx.enter_context(tc.tile_pool(name="expp", bufs=10))
        opool = ctx.enter_context(tc.tile_pool(name="op", bufs=2))
        smp = ctx.enter_context(tc.tile_pool(name="smp", bufs=2))

        # ---- small constants first (cheap DMAs), then x blocks in
        # consumption order: xk asc, xq desc (Q tile 3 first), xv asc ----
        wk = cpool.tile([128, 8, EL], F16, tag="wk")
        nc.sync.dma_start(wk[:], wk_d.ap().rearrange("(k p) m -> p k m", p=128))
        bkt = cpool.tile([128, 2], F32, tag="bkt")
        nc.sync.dma_start(bkt[:], bk_d.ap().rearrange("(c p) -> p c", p=128))
        wq = cpool.tile([128, 8, EL], F16, tag="wq")
        nc.sync.dma_start(wq[:], wq_d.ap().rearrange("(k p) m -> p k m", p=128))
        bqt = cpool.tile([128, 2], F32, tag="bqt")
        nc.sync.dma_start(bqt[:], bq_d.ap().rearrange("(c p) -> p c", p=128))
        wv = cpool.tile([128, 8, EL], F16, tag="wv")
        nc.sync.dma_start(wv[:], wv_d.ap().rearrange("(k p) m -> p k m", p=128))
        bvr = cpool.tile([1, EL], F32, tag="bvr")
        nc.sync.dma_start(bvr[:], bv_d.ap().rearrange("(p m) -> p m", p=1))
        bvb = cpool.tile([128, EL], F32, tag="bvb")
        nc.gpsimd.partition_broadcast(bvb[:], bvr[:])
        mk2 = cpool.tile([128, 2, 128], F16, tag="mk2")
        nc.sync.dma_start(mk2[:], mask_d.ap())

        VP = cpool.tile([128, 16, 4 * 66], F16, tag="VP")  # 66: 4B-aligned blocks
        # ones columns of V' (col 64 of each 66-block)
        nc.sync.dma_start(
            VP[:].rearrange("p k (h x) -> p k h x", h=4)[:, :, :, 64:65],
            vones_d.ap(),
        )

        xk = cpool.tile([128, 8, S], F16, tag="xk")
        for tb in range(4):
            nc.sync.dma_start(
                xk[:, :, tb * 512:(tb + 1) * 512],
                xk_d.ap().rearrange("(k p) m -> p k m", p=128)[
                    :, :, tb * 512:(tb + 1) * 512],
            )
        xq = cpool.tile([128, 8, S], F16, tag="xq")
        for tb in range(3, -1, -1):
            nc.sync.dma_start(
                xq[:, :, tb * 512:(tb + 1) * 512],
                xq_d.ap().rearrange("(k p) m -> p k m", p=128)[
                    :, :, tb * 512:(tb + 1) * 512],
            )
        xv = cpool.tile([128, 8, S], F16, tag="xv")
        for tb in range(4):
            nc.sync.dma_start(
                xv[:, :, tb * 512:(tb + 1) * 512],
                xv_d.ap().rearrange("(k p) m -> p k m", p=128)[
                    :, :, tb * 512:(tb + 1) * 512],
            )
        wo = cpool.tile([128, 2, E], F16, tag="wo")
        nc.sync.dma_start(wo[:], wo_d.ap().rearrange("(c p) m -> p c m", p=128))

        KT = cpool.tile([128, 2, S], F16, tag="KT")
        QT = cpool.tile([128, 2, S], F16, tag="QT")
        valsT = cpool.tile([128, 2, S], F16, tag="valsT")

        # ---- K projection, t-block order (follows the chunked xk DMA) ----
        for tb in range(4):
            for c in range(2):
                ps = psp.tile([128, 512], F32, tag="lg", bufs=2,
                              name=f"kps{tb}_{c}")
                for k in range(8):
                    nc.tensor.matmul(
                        ps[:],
                        lhsT=wk[:, k, c * 128:(c + 1) * 128],
                        rhs=xk[:, k, tb * 512:(tb + 1) * 512],
                        start=(k == 0), stop=(k == 7))
                nc.vector.tensor_scalar_add(
                    KT[:, c, tb * 512:(tb + 1) * 512], ps[:], bkt[:, c:c + 1])

        def qproj_tt(tt, tag):
            for c in range(2):
                ps = psp.tile([128, 512], F32, tag=tag,
                              bufs=2 if tag == "lg" else 1,
                              name=f"qps{tt}_{c}")
                for k in range(8):
                    nc.tensor.matmul(
                        ps[:],
                        lhsT=wq[:, k, c * 128:(c + 1) * 128],
                        rhs=xq[:, k, tt * 512:(tt + 1) * 512],
                        start=(k == 0), stop=(k == 7))
                nc.vector.tensor_scalar_add(
                    QT[:, c, tt * 512:(tt + 1) * 512], ps[:], bqt[:, c:c + 1])

        qproj_tt(3, "lg")  # critical path: first exp needs Q tile 3

        # ---- V projection tile (woven into qt=3 attention) ----
        def vproj_tile(t3):
            ps = psp.tile([128, EL], F32, tag="ops", bufs=1, name=f"vps{t3}")
            for k in range(8):
                nc.tensor.matmul(
                    ps[:],
                    lhsT=xv[:, k, t3 * 128:(t3 + 1) * 128],
                    rhs=wv[:, k, :],
                    start=(k == 0), stop=(k == 7))
            nc.vector.tensor_add(
                VP[:, t3, :].rearrange("p (h x) -> p h x", h=4)[:, :, 0:64],
                ps[:].rearrange("p (h x) -> p h x", h=4),
                bvb[:].rearrange("p (h x) -> p h x", h=4))

        # ---- O projection for one 128-row output chunk ----
        def oproj_tt(tt):
            ops = psp.tile([128, 2, 512], F32, tag="ops", bufs=1,
                           name=f"ops{tt}")
            for eo in range(2):
                for c in range(2):
                    nc.tensor.matmul(
                        ops[:, eo, :],
                        lhsT=valsT[:, c, tt * 128:(tt + 1) * 128],
                        rhs=wo[:, c, eo * 512:(eo + 1) * 512],
                        start=(c == 0), stop=(c == 1))
            ot = opool.tile([128, 2, 512], F16, tag="ot", name=f"ot{tt}")
            nc.vector.tensor_copy(ot[:], ops[:])
            nc.sync.dma_start(
                out_d.ap()[tt * 128:(tt + 1) * 128, :],
                ot[:].rearrange("p a b -> p (a b)"))

        # ---- attention + O-projection, software-pipelined across q-tiles.
        # Boundary work (128-contraction matmuls) per (qt, c, batch):
        # qt=3 interleaves the remaining Q-proj tiles; qt<3 interleaves the
        # previous q-tile's O-projection chunks.
        boundary = {
            (3, 0, 0): lambda: qproj_tt(2, "ops"),
            (3, 0, 8): lambda: qproj_tt(1, "ops"),
            (3, 1, 0): lambda: qproj_tt(0, "ops"),
            (2, 0, 0): lambda: (oproj_tt(12), oproj_tt(13)),
            (2, 0, 8): lambda: (oproj_tt(14), oproj_tt(15)),
            (1, 0, 0): lambda: (oproj_tt(8), oproj_tt(9)),
            (1, 1, 0): lambda: (oproj_tt(10), oproj_tt(11)),
            (0, 0, 0): lambda: (oproj_tt(4), oproj_tt(5)),
            (0, 1, 0): lambda: (oproj_tt(6), oproj_tt(7)),
        }

        for qt in range(3, -1, -1):
            nkt = 4 * qt + 4
            accs = {}
            for c in range(2):
                for hh in range(2):
                    accs[(c, hh)] = psp.tile([65, 512], F32, tag="acc",
                                             bufs=2, name=f"acc{qt}_{c}_{hh}")
            exs = {}

            def lg_exp(c, kt):
                dd = kt * 128 - qt * 512
                s = max(dd, 0)
                lg = psp.tile([128, 2, 512], F32, tag="lg", bufs=2,
                              name=f"lg{qt}_{c}_{kt}")
                for hh in range(2):
                    nc.tensor.matmul(
                        lg[:, hh, s:512],
                        lhsT=KT[hh * 64:(hh + 1) * 64, c,
                                kt * 128:(kt + 1) * 128],
                        rhs=QT[hh * 64:(hh + 1) * 64, c,
                               qt * 512 + s:(qt + 1) * 512],
                        start=True, stop=True)
                ex = expp.tile([128, 2, 512], F16, tag="ex",
                               name=f"ex{qt}_{c}_{kt}")
                nc.scalar.activation(ex[:, :, s:512], lg[:, :, s:512], AF.Exp,
                                     scale=0.125)
                if dd >= 0:  # diagonal tile: lower-tri mask on first 128 cols
                    nc.vector.tensor_mul(ex[:, :, s:s + 128],
                                         ex[:, :, s:s + 128], mk2[:])
                exs[(c, kt)] = ex

            def attn_v(c, kt):
                ex = exs.pop((c, kt))
                s = max(kt * 128 - qt * 512, 0)
                for hh in range(2):
                    h = 2 * c + hh
                    nc.tensor.matmul(
                        accs[(c, hh)][:, s:512],
                        lhsT=VP[:, kt, h * 66:h * 66 + 65],
                        rhs=ex[:, hh, s:512],
                        start=(kt == 0), stop=(kt == nkt - 1),
                        skip_group_check=True)

            for c in range(2):
                pend = 0
                for b0 in range(0, nkt, 8):
                    bend = min(b0 + 8, nkt)
                    for kt in range(b0, bend):
                        lg_exp(c, kt)
                    bw = boundary.get((qt, c, b0))
                    if bw is not None:
                        bw()
                    while pend <= bend - 3:
                        if qt == 3 and c == 0:
                            vproj_tile(pend)
                        attn_v(c, pend)
                        pend += 1
                while pend < nkt:
                    if qt == 3 and c == 0:
                        vproj_tile(pend)
                    attn_v(c, pend)
                    pend += 1

                # normalize chunk c: row 64 of acc is the softmax denominator
                for hh in range(2):
                    rr = smp.tile([128, 512], F32, tag="rr",
                                  name=f"rr{qt}_{c}_{hh}")
                    nc.vector.reciprocal(rr[0:1, :], accs[(c, hh)][64:65, :])
                    bc = smp.tile([128, 512], F32, tag="bc",
                                  name=f"bc{qt}_{c}_{hh}")
                    nc.gpsimd.partition_broadcast(bc[0:64, :], rr[0:1, :])
                    nc.vector.tensor_mul(
                        valsT[hh * 64:(hh + 1) * 64, c,
                              qt * 512:(qt + 1) * 512],
                        accs[(c, hh)][0:64, :], bc[0:64, :])

        # tail: first q-tile's O-projection
        for tt in range(4):
            oproj_tt(tt)

    nc.compile()
    return nc


def get_nc():
    if "nc" not in _CACHE:
        _CACHE["nc"] = _build()
    return _CACHE["nc"]


def _masks():
    i = np.arange(128)[:, None]
    j = np.arange(128)[None, :]
    m = (i <= j).astype(NP16)  # within-window causal: keep k <= q
    return np.broadcast_to(m[:, None, :], (128, 2, 128)).copy()


def make_in_maps(query, key, value, Wq, bq, Wk, bk, Wv, bv, Wo, bo):
    query = np.asarray(query, np.float32)
    key = np.asarray(key, np.float32)
    value = np.asarray(value, np.float32)
    Wq, Wk, Wv, Wo = (np.asarray(a, np.float32) for a in (Wq, Wk, Wv, Wo))
    bq, bk, bv = (np.asarray(a, np.float32) for a in (bq, bk, bv))
    masks = _masks()
    vones = np.ones((128, 16, 4, 1), NP16)
    in_maps = []
    for c in range(NCORES):
        b, g = divmod(c, 4)
        sl = slice(g * EL, (g + 1) * EL)
        in_maps.append({
            "xqT": np.ascontiguousarray(query[b].T).astype(NP16),
            "xkT": np.ascontiguousarray(key[b].T).astype(NP16),
            "xvT": np.ascontiguousarray(value[b].T).astype(NP16),
            "wqT": np.ascontiguousarray(Wq[sl, :].T).astype(NP16),
            "wkT": np.ascontiguousarray(Wk[sl, :].T).astype(NP16),
            "wvT": np.ascontiguousarray(Wv[sl, :].T).astype(NP16),
            "woT": np.ascontiguousarray(Wo[:, sl].T).astype(NP16),
            "bq": np.ascontiguousarray(bq[sl]),
            "bk": np.ascontiguousarray(bk[sl]),
            "bv": np.ascontiguousarray(bv[sl]),
            "vones": vones,
            "masks": masks,
        })
    return in_maps


def run(inputs, trace=False, tmpdir=None):
    """Run on 8 cores; returns (full_output, BassKernelResults)."""
    nc = get_nc()
    in_maps = make_in_maps(**inputs)
    res = bass_utils.run_bass_kernel_spmd(
        nc, in_maps, list(range(NCORES)), trace=trace, tmpdir=tmpdir)
    bo = np.asarray(inputs["bo"], np.float32)
    out = np.zeros((B, S, E), np.float32)
    for c in range(NCORES):
        out[c // 4] += res.results[c]["out"]
    out += bo[None, None, :]
    return out, res


def kernel(**inputs):
    out, _ = run(inputs)
    return out
